# revision 33
# baseline (speedup 1.0000x reference)
"""Trainium2 Bass kernel for nn_MultiHeadEntityOPTAttention.

Multi-head attention with sparsemax over scores + entity-select combine.
Data-parallel over batch: 32 batches -> 8 NeuronCores x 4 batches, no
collectives.

v2 design:
- fp16 everywhere on the matmul paths (PSUM accumulation stays fp32);
  fp16 weight loads trigger FWL (2x faster LDWEIGHTS) and fp16 SBUF
  operands unlock DVE 2x modes.
- k projection eliminated: A_h = Wq_h @ Wk_h^T precomputed once on-chip,
  scores = (x @ A_h / temp) @ x^T, removing 8 PSUM->SBUF copies and 32
  matmuls per batch.
- sparsemax tau via 3 exact-Newton (Michelot) updates: 4 relu passes
  (pass0 from PSUM on ACT, pass1/pass2/final on DVE) + 3 count passes
  on GpSimd (which is otherwise idle), normalization by the achieved
  row-sum hides the residual tau error.
- select-combine + mean-over-heads folded into the attn transpose as a
  diagonal matmul (baseline trick), diag built on DVE.

Self-contained: hardcodes all shapes; builds the Bass program once per
process and runs it SPMD on cores 0..7 via run_bass_kernel_spmd.
"""
import numpy as np
from contextlib import ExitStack

import concourse.bass as bass
import concourse.tile as tile
import concourse.mybir as mybir
from concourse import bacc
from concourse.masks import make_identity

F32 = mybir.dt.float32
F16 = mybir.dt.float16
U8 = mybir.dt.uint8
AF = mybir.ActivationFunctionType
ALU = mybir.AluOpType
ts = bass.ts
ds = bass.ds

B, T, E, NH, NA = 32, 256, 256, 8, 64
NCORES = 8
BPC = B // NCORES          # batches per core
P = 128
QT = T // P                # 2 partition tiles along q
ET = E // P                # 2 tiles along e (contraction)
NEG_BIG = -60000.0         # fp16-safe mask offset

# engine assignment knobs
COUNT_ON_GPSIMD = True
MASK_ON_GPSIMD = True


def build_nc():
    nc = bacc.Bacc("TRN2", target_bir_lowering=False, debug=False,
                   num_devices=NCORES)
    x_d = nc.dram_tensor("x", [BPC, T, E], F32, kind="ExternalInput").ap()
    mask_d = nc.dram_tensor("mask", [BPC, T, T], U8, kind="ExternalInput").ap()
    A_d = nc.dram_tensor("w_A", [E, NH * E], F32, kind="ExternalInput").ap()
    wv_d = nc.dram_tensor("w_v", [E, NH * E], F32, kind="ExternalInput").ap()
    fsw_d = nc.dram_tensor("fc_select_w", [E, NH], F32, kind="ExternalInput").ap()
    fsb_d = nc.dram_tensor("fc_select_b", [1, NH], F32, kind="ExternalInput").ap()
    out_d = nc.dram_tensor("out", [BPC, T, E], F32, kind="ExternalOutput").ap()

    with tile.TileContext(nc) as tc, ExitStack() as ctx:
        const_pool = ctx.enter_context(tc.tile_pool(name="const", bufs=1))
        w_pool = ctx.enter_context(tc.tile_pool(name="weights", bufs=1))
        x_pool = ctx.enter_context(tc.tile_pool(name="x", bufs=2))
        mask_pool = ctx.enter_context(tc.tile_pool(name="mask", bufs=2))
        qk_pool = ctx.enter_context(tc.tile_pool(name="qk", bufs=2))
        v_pool = ctx.enter_context(tc.tile_pool(name="v", bufs=3))
        u0_pool = ctx.enter_context(tc.tile_pool(name="u0", bufs=2))
        attn_pool = ctx.enter_context(tc.tile_pool(name="attn", bufs=2))
        attnT_pool = ctx.enter_context(tc.tile_pool(name="attnT", bufs=4))
        diag_pool = ctx.enter_context(tc.tile_pool(name="diag", bufs=2))
        uscr_pool = ctx.enter_context(tc.tile_pool(name="uscr", bufs=8))
        stats_pool = ctx.enter_context(tc.tile_pool(name="stats", bufs=3))
        sel_pool = ctx.enter_context(tc.tile_pool(name="sel", bufs=3))
        outf_pool = ctx.enter_context(tc.tile_pool(name="outf", bufs=2))

        mm_ps = ctx.enter_context(tc.tile_pool(name="mmps", bufs=2, space="PSUM"))
        pp_ps = ctx.enter_context(tc.tile_pool(name="ppps", bufs=1, space="PSUM"))
        sc_ps = ctx.enter_context(tc.tile_pool(name="scps", bufs=2, space="PSUM"))
        h0_ps = ctx.enter_context(tc.tile_pool(name="h0ps", bufs=1, space="PSUM"))
        mean_ps = ctx.enter_context(tc.tile_pool(name="meanps", bufs=1, space="PSUM"))

        # ---- constants / weights ----------------------------------------
        identh = const_pool.tile([P, P], F16)
        make_identity(nc, identh[:])
        identf = const_pool.tile([P, P], F32)
        make_identity(nc, identf[:])
        ones_row = const_pool.tile([1, NA], F16)
        nc.vector.memset(ones_row[:], 1.0)
        zeroh = const_pool.tile([P, T], F16)
        nc.vector.memset(zeroh[:], 0.0)
        ntprobe = const_pool.tile([P, 1], F32)
        nc.vector.memset(ntprobe[:], -0.3)

        wv = w_pool.tile([P, ET, NH * E], F16)
        A_sb = w_pool.tile([P, ET, NH * E], F16)
        fsw = const_pool.tile([P, ET, NH], F16)
        fsb = const_pool.tile([1, NH], F16)


        def prep(b):
            """loads + mask prep + xT + select softmax inputs."""
            S = {'b': b}
            x_nat = x_pool.tile([P, QT, E], F16, tag="xnat")
            nc.gpsimd.dma_start(x_nat[:], x_d[b].rearrange("(i p) e -> p i e", p=P))
            mask_u8 = mask_pool.tile([P, QT, T], U8, tag="m8")
            nc.sync.dma_start(mask_u8[:], mask_d[b].rearrange("(i p) k -> p i k", p=P))

            eng_m = nc.gpsimd if MASK_ON_GPSIMD else nc.vector
            maskneg = mask_pool.tile([P, QT, T], F16, tag="mneg")
            nc.vector.tensor_scalar_mul(maskneg[:], mask_u8[:], NEG_BIG)

            rowsum = stats_pool.tile([P, QT], F32, tag="rowsum")
            for qt in range(QT):
                nc.vector.tensor_reduce(rowsum[:, qt:qt + 1], mask_u8[:, qt, :],
                                        axis=mybir.AxisListType.X, op=ALU.add)
            notrow = stats_pool.tile([P, QT], F32, tag="notrow")
            eng_m.tensor_scalar(out=notrow[:], in0=rowsum[:],
                                scalar1=float(T) - 0.5, scalar2=None,
                                op0=ALU.is_lt)
            S['notrow'] = notrow

            xT = x_pool.tile([P, ET, T], F16, tag="xT")
            xtp = mm_ps.tile([P, 2 * T], F16, tag="mm")
            for i in range(QT):
                for j in range(ET):
                    nc.tensor.transpose(xtp[:, ds(j * T + i * P, P)],
                                        x_nat[:, i, ts(j, P)], identh[:])
            nc.vector.tensor_copy(xT[:], xtp[:].rearrange("p (i t) -> p i t", i=ET))

            notmask = sel_pool.tile([NA, T], F16, tag="nm")
            nc.vector.tensor_scalar(out=notmask[:], in0=mask_u8[0:NA, 0, :],
                                scalar1=-1.0, scalar2=1.0,
                                op0=ALU.mult, op1=ALU.add)
            notmaskT = sel_pool.tile([P, QT, NA], F16, tag="nmT")
            nmp = mm_ps.tile([P, QT, NA], F16, tag="mm")
            for i in range(QT):
                nc.tensor.transpose(nmp[:, i, :], notmask[:, ts(i, P)],
                                    identh[0:NA, 0:NA])
            nc.vector.tensor_copy(notmaskT[:], nmp[:])

            xat = sel_pool.tile([P, ET, NA], F16, tag="xat")
            xatp = mm_ps.tile([P, ET, NA], F32, tag="mm")
            for j in range(ET):
                for i in range(QT):
                    nc.tensor.matmul(xatp[:, j, :], x_nat[:, i, ts(j, P)],
                                     notmaskT[:, i, :],
                                     start=(i == 0), stop=(i == QT - 1))
            nc.vector.tensor_copy(xat[:], xatp[:])

            logits = mm_ps.tile([NA, NH], F32, tag="mm")
            for j in range(ET):
                nc.tensor.matmul(logits[:], xat[:, j, :], fsw[:, j, :],
                                 start=(j == 0), stop=False)
            nc.tensor.matmul(logits[:], ones_row[:], fsb[:],
                             start=False, stop=True)
            selmx = sel_pool.tile([NA, 1], F32, tag="selmx")
            nc.vector.tensor_reduce(selmx[:], logits[:],
                                    axis=mybir.AxisListType.X, op=ALU.max,
                                    negate=True)
            sel_exp = sel_pool.tile([NA, NH], F32, tag="selexp")
            selsum = sel_pool.tile([NA, 1], F32, tag="selsum")
            nc.scalar.activation(sel_exp[:], logits[:], AF.Exp,
                                 bias=selmx[:], scale=1.0, accum_out=selsum[:])
            selrec = sel_pool.tile([NA, 1], F32, tag="selrec")
            nc.vector.reciprocal(selrec[:], selsum[:])
            sel = sel_pool.tile([NA, NH], F32, tag="sel")
            nc.vector.tensor_scalar_mul(sel[:], sel_exp[:], selrec[:])
            S['sel'] = sel
            S['x_nat'] = x_nat
            S['xT'] = xT
            S['maskneg'] = maskneg
            return S

        def alloc_qkv(S):
            S['q'] = qk_pool.tile([P, NH, ET, T], F16, tag="qk", name="q_all")
            S['v'] = v_pool.tile([P, NH, QT, E], F16, tag="v", name="v_all")

        def s1_pair(S, g):
            """s1 = x @ A_h / 16 (q role), for heads 2g, 2g+1."""
            xT = S['xT']
            qp = pp_ps.tile([P, 2, ET, T], F32, tag="pp")
            for hh in range(2):
                h = 2 * g + hh
                for j in range(ET):
                    for i in range(ET):
                        nc.tensor.matmul(qp[:, hh, j, :],
                                         A_sb[:, i, ds(h * E + j * P, P)],
                                         xT[:, i, :], start=(i == 0),
                                         stop=(i == ET - 1))
            if S['b'] == 0 and g % 2 == 1:
                # batch 0 has no back-phase work: give DVE some copies
                nc.vector.tensor_scalar_mul(S['q'][:, ds(2 * g, 2), :, :],
                                            qp[:], 1.0 / 16.0)
            else:
                nc.scalar.activation(S['q'][:, ds(2 * g, 2), :, :], qp[:],
                                     AF.Copy, bias=0.0, scale=1.0 / 16.0)

        def v_pair(S, g):
            """v = x @ Wv_h for heads 2g, 2g+1."""
            xT = S['xT']
            vp = pp_ps.tile([P, 2, QT, E], F32, tag="pp")
            for hh in range(2):
                h = 2 * g + hh
                for i in range(QT):
                    for j in range(ET):
                        nc.tensor.matmul(vp[:, hh, i, :], xT[:, j, ts(i, P)],
                                         wv[:, j, ds(h * E, E)],
                                         start=(j == 0), stop=(j == ET - 1))
            nc.scalar.activation(S['v'][:, ds(2 * g, 2), :, :], vp[:], AF.Copy,
                                 bias=0.0, scale=1.0)

        def alloc_sparse(S):
            S['u0'] = u0_pool.tile([P, QT, NH, T], F16, tag="u0", name="u0")
            S['ntau'] = stats_pool.tile([P, QT, NH], F32, tag="ntau", name="ntau")
            S['ptau'] = stats_pool.tile([P, QT, NH], F32, tag="ptau", name="ptau")
            S['fstA'] = stats_pool.tile([P, QT, NH], F32, tag="fstA", name="fstA")
            S['fstB'] = stats_pool.tile([P, QT, NH], F32, tag="fstB", name="fstB")
            S['fstF'] = stats_pool.tile([P, QT, NH], F32, tag="fstF", name="fstF")

        def scores_chunk(S, qt, h2):
            """scores for heads (2*h2, 2*h2+1) at row tile qt + pass 0."""
            u0, ntau = S['u0'], S['ntau']
            sc = sc_ps.tile([P, 2, T], F32, tag="sc")
            for hh in range(2):
                h = h2 * 2 + hh
                nc.tensor.matmul(sc[:, hh, :], identh[:], S['maskneg'][:, qt, :],
                                 start=True, stop=False)
                for i in range(ET):
                    nc.tensor.matmul(sc[:, hh, :], S['q'][:, h, i, ts(qt, P)],
                                     S['xT'][:, i, :],
                                     start=False, stop=(i == ET - 1))
            nmx = stats_pool.tile([P, 2], F32, tag="nmx")
            nc.vector.tensor_reduce(nmx[:], sc[:],
                                    axis=mybir.AxisListType.X, op=ALU.max,
                                    negate=True)
            # ntau0 = 1 - rowmax  (bias for pass0: u0 = relu(sc + ntau0))
            nc.vector.tensor_scalar(
                out=ntau[:, qt, ts(h2, 2)], in0=nmx[:],
                scalar1=1.0, scalar2=None, op0=ALU.add)
            for hh in range(2):
                h = h2 * 2 + hh
                if S['b'] == 0 and hh == 1:
                    nc.vector.scalar_tensor_tensor(
                        out=u0[:, qt, h, :], in0=sc[:, hh, :],
                        scalar=ntau[:, qt, h:h + 1], in1=zeroh[:],
                        op0=ALU.add, op1=ALU.max,
                        accum_out=S['fstA'][:, qt, h:h + 1])
                else:
                    nc.scalar.activation(u0[:, qt, h, :], sc[:, hh, :], AF.Relu,
                                         bias=ntau[:, qt, h:h + 1], scale=1.0,
                                         accum_out=S['fstA'][:, qt, h:h + 1])

        TPROBE = 0.3

        def probe_tile(S, qt, h, on_act=False):
            """fstB[qt,h] = sum relu(u0 - TPROBE)."""
            scr = uscr_pool.tile([P, T], F16, tag="uscr")
            if on_act:
                nc.scalar.activation(scr[:], S['u0'][:, qt, h, :], AF.Relu,
                                     bias=ntprobe[:], scale=1.0,
                                     accum_out=S['fstB'][:, qt, h:h + 1])
            else:
                nc.vector.scalar_tensor_tensor(
                    out=scr[:], in0=S['u0'][:, qt, h, :],
                    scalar=-TPROBE, in1=zeroh[:],
                    op0=ALU.add, op1=ALU.max,
                    accum_out=S['fstB'][:, qt, h:h + 1])

        def secant_init(S):
            """chord through (0, f0a), (TPROBE, f0b): t1 = .3 - .3*(f0b-1)/df."""
            d_f = stats_pool.tile([P, QT, NH], F32, tag="d_f")
            nc.vector.scalar_tensor_tensor(
                out=d_f[:], in0=S['fstB'][:], scalar=-1e-3,
                in1=S['fstA'][:], op0=ALU.add, op1=ALU.subtract)
            rec = stats_pool.tile([P, QT, NH], F32, tag="srec")
            nc.vector.reciprocal(rec[:], d_f[:])
            u = stats_pool.tile([P, QT, NH], F32, tag="sa")
            nc.vector.scalar_tensor_tensor(
                out=u[:], in0=S['fstB'][:], scalar=-1.0,
                in1=rec[:], op0=ALU.add, op1=ALU.mult)
            nc.vector.tensor_scalar(out=S['ptau'][:], in0=u[:],
                                    scalar1=-TPROBE, scalar2=TPROBE,
                                    op0=ALU.mult, op1=ALU.add)
            nc.vector.tensor_scalar(out=S['ptau'][:], in0=S['ptau'][:],
                                    scalar1=0.0, scalar2=0.999,
                                    op0=ALU.max, op1=ALU.min)
            nc.vector.tensor_scalar_mul(S['ntau'][:], S['ptau'][:], -1.0)
            dt0 = stats_pool.tile([P, QT, NH], F32, tag="step")
            nc.vector.tensor_scalar(out=dt0[:], in0=S['ptau'][:],
                                    scalar1=-TPROBE, scalar2=None,
                                    op0=ALU.add)
            S['dt'] = dt0

        def secant_update(S, f_new, f_old):
            """step = (1 - f_new)*dt/(f_new - f_old - eps); t += step, clamped."""
            d_f = stats_pool.tile([P, QT, NH], F32, tag="d_f")
            nc.vector.scalar_tensor_tensor(
                out=d_f[:], in0=S[f_new][:], scalar=-1e-3,
                in1=S[f_old][:], op0=ALU.add, op1=ALU.subtract)
            rec = stats_pool.tile([P, QT, NH], F32, tag="srec")
            nc.vector.reciprocal(rec[:], d_f[:])
            a = stats_pool.tile([P, QT, NH], F32, tag="sa")
            nc.vector.scalar_tensor_tensor(
                out=a[:], in0=S[f_new][:], scalar=-1.0,
                in1=rec[:], op0=ALU.add, op1=ALU.mult)
            step = stats_pool.tile([P, QT, NH], F32, tag="step")
            nc.vector.scalar_tensor_tensor(
                out=step[:], in0=a[:], scalar=-1.0,
                in1=S['dt'][:], op0=ALU.mult, op1=ALU.mult)
            tnew = stats_pool.tile([P, QT, NH], F32, tag="ptau2")
            nc.vector.tensor_tensor(out=tnew[:], in0=S['ptau'][:],
                                    in1=step[:], op=ALU.add)
            nc.vector.tensor_scalar(out=tnew[:], in0=tnew[:],
                                    scalar1=0.0, scalar2=0.999,
                                    op0=ALU.max, op1=ALU.min)
            dtn = stats_pool.tile([P, QT, NH], F32, tag="dtn")
            nc.vector.tensor_tensor(out=dtn[:], in0=tnew[:],
                                    in1=S['ptau'][:], op=ALU.subtract)
            nc.vector.tensor_scalar_mul(S['ntau'][:], tnew[:], -1.0)
            S['ptau'] = tnew
            S['dt'] = dtn

        def pass_tile(S, qt, h, fdst, last, on_act=False):
            """relu(u0 - ptau) with row-sum accum; final pass writes attn."""
            if last:
                out_ap = S['attn'][:, qt, h, :]
            else:
                scr = uscr_pool.tile([P, T], F16, tag="uscr")
                out_ap = scr[:]
            if on_act:
                nc.scalar.activation(out_ap, S['u0'][:, qt, h, :], AF.Relu,
                                     bias=S['ntau'][:, qt, h:h + 1], scale=1.0,
                                     accum_out=S[fdst][:, qt, h:h + 1])
            else:
                nc.vector.scalar_tensor_tensor(
                    out=out_ap, in0=S['u0'][:, qt, h, :],
                    scalar=S['ntau'][:, qt, h:h + 1], in1=zeroh[:],
                    op0=ALU.add, op1=ALU.max,
                    accum_out=S[fdst][:, qt, h:h + 1])

        def newton_piece(S, piece):
            """8 interleavable pieces of the back-phase iteration."""
            last_b = S['b'] == BPC - 1
            pr_act = (3, 7, 11, 15) if last_b else (5, 10, 15)
            mid_act = ((2, 5, 7, 10, 13, 15) if last_b
                       else (2, 5, 7))
            mid_act2 = ((2, 5, 7, 10, 13, 15) if last_b
                        else (10, 13, 15))
            if piece == 0:
                for idx in range(8):
                    probe_tile(S, idx // NH, idx % NH, on_act=(idx in pr_act))
            elif piece == 1:
                for t in range(8):
                    idx = 8 + t
                    probe_tile(S, idx // NH, idx % NH, on_act=(idx in pr_act))
                secant_init(S)
            elif piece == 2:
                for idx in range(8):
                    pass_tile(S, idx // NH, idx % NH, 'fstA', last=False,
                              on_act=(idx in mid_act))
            elif piece == 3:
                for t in range(8):
                    idx = 8 + t
                    pass_tile(S, idx // NH, idx % NH, 'fstA', last=False,
                              on_act=(idx in mid_act2))
                secant_update(S, 'fstA', 'fstB')
            elif piece == 4:
                for idx in range(8):
                    pass_tile(S, idx // NH, idx % NH, 'fstB', last=False,
                              on_act=(idx in mid_act))
            elif piece == 5:
                for t in range(8):
                    idx = 8 + t
                    pass_tile(S, idx // NH, idx % NH, 'fstB', last=False,
                              on_act=(idx in mid_act2))
                secant_update(S, 'fstB', 'fstA')
            elif piece == 6:
                S['attn'] = attn_pool.tile([P, QT, NH, T], F16, tag="attn",
                                           name="attn")
                for t in range(8):
                    pass_tile(S, t // NH, t % NH, 'fstF', last=True,
                              on_act=(t % 2 == 1))
            else:
                for t in range(8):
                    idx = 8 + t
                    pass_tile(S, idx // NH, idx % NH, 'fstF', last=True,
                              on_act=(idx % 2 == 1))

        def normalize(S):
            recipf = stats_pool.tile([P, QT, NH], F32, tag="recipf")
            nc.vector.reciprocal(recipf[:], S['fstF'][:])
            for qt in range(QT):
                nc.vector.tensor_scalar_mul(recipf[:, qt, :], recipf[:, qt, :],
                                            S['notrow'][:, qt:qt + 1])
            # dall[p, qt, h]: transpose-diag scales. Rows 0..63 of qt=0 get
            # recipf*sel (select-combine fold); all mean rows get recipf/NH.
            dall = stats_pool.tile([P, QT, NH], F32, tag="dall")
            nc.vector.tensor_tensor(out=dall[0:NA, 0, :], in0=recipf[0:NA, 0, :],
                                    in1=S['sel'][:], op=ALU.mult)
            nc.vector.tensor_scalar_mul(dall[ds(NA, NA), 0, :],
                                        recipf[ds(NA, NA), 0, :], 1.0 / NH)
            nc.vector.tensor_scalar_mul(dall[:, 1, :], recipf[:, 1, :], 1.0 / NH)
            S['dall'] = dall
            S['outf'] = outf_pool.tile([P, QT, E], F32, tag="outf", name="outf")
            S['outA'] = h0_ps.tile([P, E], F32, tag="h0", name="outA")
            S['outB'] = mean_ps.tile([P, E], F32, tag="mean", name="outB")

        def out_head(S, h):
            attn, v_all = S['attn'], S['v']
            attnT = attnT_pool.tile([P, QT, T], F16, tag="attnT")
            atp = mm_ps.tile([P, QT, T], F16, tag="mm")
            # scale attn rows by dall (select/mean fold) then pure transpose
            attnS = diag_pool.tile([P, QT, T], F16, tag="attnS")
            for qt in range(QT):
                nc.vector.tensor_scalar_mul(attnS[:, qt, :],
                                            attn[:, qt, h, :],
                                            S['dall'][:, qt, h:h + 1])
            for ki in range(QT):
                for qt in range(QT):
                    nc.tensor.transpose(atp[:, ki, ts(qt, P)],
                                        attnS[:, qt, ts(ki, P)], identh[:])
            if h % 2 == 0:
                nc.scalar.activation(attnT[:], atp[:], AF.Copy,
                                     bias=0.0, scale=1.0)
            else:
                nc.vector.tensor_copy(attnT[:], atp[:])
            for ki in range(QT):
                nc.tensor.matmul(S['outA'][:], attnT[:, ki, ts(0, P)],
                                 v_all[:, h, ki, :],
                                 start=(h == 0 and ki == 0),
                                 stop=(h == NH - 1 and ki == QT - 1))
            for ki in range(QT):
                nc.tensor.matmul(S['outB'][:], attnT[:, ki, ts(1, P)],
                                 v_all[:, h, ki, :],
                                 start=(h == 0 and ki == 0),
                                 stop=(h == NH - 1 and ki == QT - 1))

        def finish(b, S):
            nc.vector.tensor_copy(S['outf'][:, 0, :], S['outA'][:])
            nc.scalar.activation(S['outf'][:, 1, :], S['outB'][:],
                                 AF.Copy, bias=0.0, scale=1.0)
            nc.sync.dma_start(out_d[b].rearrange("(i p) e -> p i e", p=P),
                              S['outf'][:])

        # ---- skewed + interleaved pipeline ------------------------------
        # phase 1: s1 heads + score chunks + pass0 (front, PE/ACT-dense)
        #          interleaved with the newton iteration (back, DVE-dense)
        # phase 2: v heads (front) interleaved with attn-transpose + output
        #          matmuls (back) - PE stays busy in both phases (HAM warm)
        st = [None] * BPC
        st[0] = prep(0)
        alloc_qkv(st[0])
        alloc_sparse(st[0])
        # weight loads issued after batch-0's x/mask DMAs; per-head slices so
        # the first s1 head can start as soon as its A slice lands
        for h in range(NH):
            nc.gpsimd.dma_start(
                A_sb[:, :, ds(h * E, E)],
                A_d[:, ds(h * E, E)].rearrange("(i p) n -> p i n", p=P))
        for h in range(NH):
            nc.gpsimd.dma_start(
                wv[:, :, ds(h * E, E)],
                wv_d[:, ds(h * E, E)].rearrange("(i p) n -> p i n", p=P))
        nc.gpsimd.dma_start(fsw[:], fsw_d.rearrange("(i p) n -> p i n", p=P))
        nc.gpsimd.dma_start(fsb[:], fsb_d)
        for s in range(BPC + 2):
            F = s < BPC                    # front batch s
            N = 1 <= s <= BPC              # newton for batch s-1
            O = 2 <= s <= BPC + 1          # outputs for batch s-2
            # phase 1: 12 front units (4 s1 pairs + 8 score chunks), 8 back
            for u in range(8):
                if F and u % 2 == 0:
                    s1_pair(st[s], u // 2)
                if N:
                    newton_piece(st[s - 1], u)
                if F and u % 2 == 1:
                    h2 = u // 2
                    scores_chunk(st[s], 0, h2)
                    scores_chunk(st[s], 1, h2)
            # prep the NEXT batch here so its DMAs + gpsimd mask work get a
            # full phase of runway before phase 1 of iter s+1 needs them
            if s + 1 < BPC:
                st[s + 1] = prep(s + 1)
                alloc_qkv(st[s + 1])
                alloc_sparse(st[s + 1])
            # phase 2: v heads (front) + outputs (back, batch s-2)
            if O:
                normalize(st[s - 2])
            for h in range(NH):
                if F and h % 2 == 0:
                    v_pair(st[s], h // 2)
                if O:
                    out_head(st[s - 2], h)
            if O:
                finish(s - 2, st[s - 2])
                st[s - 2] = None

    nc.compile()
    return nc


_NC_CACHE = None


def _get_nc():
    global _NC_CACHE
    if _NC_CACHE is None:
        _NC_CACHE = build_nc()
    return _NC_CACHE


def make_in_maps(x, mask, w_q, w_k, w_v, fc_select_w, fc_select_b):
    mask_u8 = np.ascontiguousarray(mask).view(np.uint8)
    # A_h = Wq_h @ Wk_h^T precomputed on host (fp16-rounded operands to
    # match the on-chip numerics the kernel was validated against)
    wq16 = w_q.astype(np.float16).astype(np.float32)
    wk16 = w_k.astype(np.float16).astype(np.float32)
    A = np.zeros((E, NH * E), np.float32)
    for hh in range(NH):
        A[:, hh * E:(hh + 1) * E] = (
            wq16[:, hh * E:(hh + 1) * E] @ wk16[:, hh * E:(hh + 1) * E].T)
    in_maps = []
    for c in range(NCORES):
        sl = slice(c * BPC, (c + 1) * BPC)
        in_maps.append({
            "x": np.ascontiguousarray(x[sl], dtype=np.float32),
            "mask": np.ascontiguousarray(mask_u8[sl]),
            "w_A": np.ascontiguousarray(A, dtype=np.float32),
            "w_v": np.ascontiguousarray(w_v, dtype=np.float32),
            "fc_select_w": np.ascontiguousarray(fc_select_w, dtype=np.float32),
            "fc_select_b": np.ascontiguousarray(
                fc_select_b, dtype=np.float32).reshape(1, NH),
        })
    return in_maps


def kernel(x, h, mask, w_q, w_k, w_v, fc_select_w, fc_select_b, **kwargs):
    from concourse import bass_utils
    nc = _get_nc()
    in_maps = make_in_maps(x, mask, w_q, w_k, w_v, fc_select_w, fc_select_b)
    res = bass_utils.run_bass_kernel_spmd(nc, in_maps,
                                          core_ids=list(range(NCORES)))
    out = np.concatenate([res.results[c]["out"] for c in range(NCORES)], axis=0)
    return out.astype(np.float32)
